# revision 23
# baseline (speedup 1.0000x reference)
"""DSAttention layer for Trainium2, 8 NeuronCores.

Sharding: core c -> batch b = c//2, head-group g = c%2 (4 heads each,
e-columns 256g..256g+255 of the 512-wide head dim).  tau[b]/8 (softmax
temperature x 1/sqrt(E)) is folded into each core's Wq/bq slice on the
host; delta[b] broadcasts over the softmax axis and is shift-invariant,
so it drops out exactly.  Each core emits its head-group's partial
output projection [2048, 512]; the host sums the pair per batch and
adds (bv @ Wo + bo).

Device dataflow per core (all matmul operands fp16, fp32 PSUM accum):
  X[q|k|v] [2048,512] --PE transpose--> X^T (d-major)
  qT/kT [e 256, l 2048] = W^T @ X^T   (e on partitions)
  v     [s 2048, e 256] -> fp16 v_aug [s,65] per head (ones col -> Z)
  scoresT[s,l] = kT.T @ qT  per head, head pairs concurrent via
                 partition-offset row groups (K=64 at rows 0-63/64-127)
  E = exp(scoresT - 2) fp16  (one ACT instr per [128, 2x512] pair tile)
  attnT_aug[65,l] = v_aug.T @ E  (accumulate 16 s-chunks in PSUM;
                 row 64 = softmax denominator Z)
  normalize: 1/Z (DVE) -> broadcast across partitions via K=1 matmul
                 -> attnT[64, h, l] in SBUF
  out[l,512] = sum_h attnT_h.T @ Wo_h  (K=64, accumulate 4 heads)
"""

import numpy as np
from contextlib import ExitStack

import concourse.bass as bass
import concourse.bacc as bacc
import concourse.mybir as mybir
import concourse.tile as tile
from concourse.bass_utils import run_bass_kernel_spmd
from concourse.masks import make_identity

F32 = mybir.dt.float32
F16 = mybir.dt.float16

B, L, S, D = 4, 2048, 2048, 512
H, E = 8, 64          # full model heads / head dim
HG = 4                # heads per core (head-group)
EG = HG * E           # 256, e-columns per core
N_CORES = 8

LT = L // 128         # 16 l-tiles
ST = S // 128         # 16 s-tiles
DC = D // 128         # 4 d-chunks
LQ = 4                # l-quarters of 512
SCALE = 1.0 / np.sqrt(np.float32(E))
EXP_SHIFT = -2.0      # exp(x-2): cancels in softmax, guards fp16 overflow


def _emit(ctx: ExitStack, tc: "tile.TileContext", io: dict):
    nc = tc.nc
    mm = nc.tensor.matmul

    singles = ctx.enter_context(tc.tile_pool(name="singles", bufs=1))
    bigs = ctx.enter_context(tc.tile_pool(name="bigs", bufs=1))
    xin_pool = ctx.enter_context(tc.tile_pool(name="xin", bufs=2))
    xt_pool = ctx.enter_context(tc.tile_pool(name="xt", bufs=2))
    e_pool = ctx.enter_context(tc.tile_pool(name="eslab", bufs=3))
    z_pool = ctx.enter_context(tc.tile_pool(name="zrec", bufs=2))
    ob_pool = ctx.enter_context(tc.tile_pool(name="outsb", bufs=3))

# One PSUM pool, statically 8 banks: sc 2x2 + avpj 1x2 + small 1x2.
    # "avpj" is reused: projection accumulators (prolog) then AV accumulators
    # (attention).  "small" is reused: transposes (prolog) then zb/wo psums.
    ps = ctx.enter_context(tc.tile_pool(name="ps", bufs=2, space="PSUM"))

    # ---- constants & weights -------------------------------------------
    ident = singles.tile([128, 128], F16)
    make_identity(nc, ident)
    ones_row = singles.tile([1, 128], F16)
    nc.vector.memset(ones_row, 1.0)
    shift_col = singles.tile([128, 1], F32)
    nc.vector.memset(shift_col, EXP_SHIFT)

    wq_sb = singles.tile([128, DC, EG], F16)   # [p, c, e] = Wq[c*128+p, e]
    wk_sb = singles.tile([128, DC, EG], F16)
    wv_sb = singles.tile([128, DC, EG], F16)
    wo_sb = singles.tile([64, HG, D], F16)     # [r, h, n] = Wo[64h+r, n]
    bq_sb = singles.tile([128, 2], F32)        # [p, ec] = bq[128ec+p]
    bk_sb = singles.tile([128, 2], F32)
    nc.sync.dma_start(out=wq_sb, in_=io["wq"][:])
    nc.sync.dma_start(out=wk_sb, in_=io["wk"][:])
    nc.sync.dma_start(out=wv_sb, in_=io["wv"][:])
    nc.sync.dma_start(out=wo_sb, in_=io["wo"][:])
    nc.sync.dma_start(out=bq_sb, in_=io["bq"][:])
    nc.sync.dma_start(out=bk_sb, in_=io["bk"][:])

    # ---- big persistent SBUF tensors -----------------------------------
    qT = bigs.tile([128, 2, L], F16, tag="qT")     # [e_in_chunk, ec, l]
    kT = bigs.tile([128, 2, S], F16, tag="kT")
    v_sb = bigs.tile([128, ST, HG, 65], F16, tag="v")  # [s_in_tile, st, h, dv+1]
    attnT = bigs.tile([64, HG, L], F16, tag="attnT")   # [e_in_head, h, l]
    nc.vector.memset(v_sb, 1.0)  # col 64 stays 1.0 (ones -> Z row)

    # ---- X^T + projections ---------------------------------------------
    def load_transpose(x_dram, lc):
        """DMA X rows [512lc .. 512lc+512) and produce xt[:, c, :] =
        X^T slab [128 d, 4 l-tiles * 128] for this l-chunk."""
        xin = xin_pool.tile([128, 4, D], F32, tag="xin")
        xc = xin_pool.tile([128, 4, D], F16, tag="xc")
        rows = x_dram[lc * 512:(lc + 1) * 512, :].rearrange(
            "(i p) d -> p i d", p=128)
        nc.sync.dma_start(out=xin, in_=rows)
        for i in range(4):
            nc.vector.tensor_copy(out=xc[:, i, :], in_=xin[:, i, :])
        xt = xt_pool.tile([128, DC, 512], F16, tag="xt")
        for i in range(4):
            for c in range(DC):
                tp = ps.tile([128, 1024], F16, tag="small",
                             name=f"tp_{lc}_{i}_{c}")[:, 0:128]
                nc.tensor.transpose(tp, xc[:, i, c * 128:(c + 1) * 128], ident)
                nc.vector.tensor_copy(out=xt[:, c, i * 128:(i + 1) * 128],
                                      in_=tp)
        return xt

    def proj_qk(xt, w_sb, b_sb, dst, lc):
        # dst[:, ec, 512lc : 512lc+512] = (W.T @ X^T) + bias
        for ec in range(2):
            pp = ps.tile([128, 512], F32, tag="avpj", name=f"pp_{lc}_{ec}")
            for c in range(DC):
                mm(pp, lhsT=w_sb[:, c, ec * 128:(ec + 1) * 128],
                   rhs=xt[:, c, :], start=(c == 0), stop=(c == DC - 1))
            nc.vector.tensor_scalar_add(
                out=dst[:, ec, lc * 512:(lc + 1) * 512], in0=pp,
                scalar1=b_sb[:, ec:ec + 1])

    def proj_v(xt, lc):
        for i in range(4):
            st = lc * 4 + i
            vp = ps.tile([128, 512], F32, tag="avpj", name=f"vp_{lc}_{i}")[:, 0:EG]
            for c in range(DC):
                mm(vp, lhsT=xt[:, c, i * 128:(i + 1) * 128],
                   rhs=wv_sb[:, c, :], start=(c == 0), stop=(c == DC - 1))
            for h in range(HG):
                nc.vector.tensor_copy(out=v_sb[:, st, h, 0:64],
                                      in_=vp[:, h * 64:(h + 1) * 64])

    for lc in range(4):
        xt = load_transpose(io["xk"], lc)
        proj_qk(xt, wk_sb, bk_sb, kT, lc)
    for lc in range(4):
        xt = load_transpose(io["xq"], lc)
        proj_qk(xt, wq_sb, bq_sb, qT, lc)
    for lc in range(4):
        xt = load_transpose(io["xv"], lc)
        proj_v(xt, lc)

    # ---- attention ------------------------------------------------------
    for lq in range(LQ):
        l0 = lq * 512
        for p in range(2):                      # head pair
            av = [ps.tile([65, 512], F32, tag="avpj", name=f"av{lq}_{p}_{i}")
                  for i in range(2)]
            for j in range(ST):
                sc = ps.tile([128, 2, 512], F32, tag="sc", name=f"sc_{lq}_{p}_{j}")
                ep = e_pool.tile([128, 2, 512], F16, tag="ep")
                for hh in range(2):             # rows 0-63 / 64-127: concurrent
                    o = hh * 64
                    mm(sc[:, hh, :],
                       lhsT=kT[o:o + 64, p, j * 128:(j + 1) * 128],
                       rhs=qT[o:o + 64, p, l0:l0 + 512],
                       start=True, stop=True)
                nc.scalar.activation(out=ep, in_=sc,
                                     func=mybir.ActivationFunctionType.Exp,
                                     bias=shift_col[:, 0:1], scale=1.0)
                for hh in range(2):
                    mm(av[hh], lhsT=v_sb[:, j, 2 * p + hh, :],
                       rhs=ep[:, hh, :], start=(j == 0), stop=(j == ST - 1))
            for hh in range(2):
                h = 2 * p + hh
                zr = z_pool.tile([1, 512], F16, tag="zr")
                with nc.allow_low_precision(reason="1/Z as f16 matmul operand"):
                    nc.vector.reciprocal(zr, av[hh][64:65, :])
                zb = ps.tile([128, 512], F32, tag="small", name=f"zb_{lq}_{p}_{hh}")
                mm(zb, lhsT=ones_row, rhs=zr, start=True, stop=True)
                zb_sb = z_pool.tile([64, 512], F32, tag="zb_sb")
                nc.vector.tensor_copy(out=zb_sb, in_=zb[0:64, :])
                nc.vector.tensor_mul(out=attnT[:, h, l0:l0 + 512],
                                     in0=av[hh][0:64, :], in1=zb_sb)
        # output projection for this l-quarter
        for i in range(4):
            lt = lq * 4 + i
            op = ps.tile([128, D], F32, tag="small", name=f"op_{lq}_{i}")
            for h in range(HG):
                mm(op, lhsT=attnT[:, h, lt * 128:(lt + 1) * 128],
                   rhs=wo_sb[:, h, :], start=(h == 0), stop=(h == HG - 1))
            ob = ob_pool.tile([128, D], F32, tag="ob")
            nc.vector.tensor_copy(out=ob, in_=op)
            nc.sync.dma_start(out=io["out"][lt * 128:(lt + 1) * 128, :], in_=ob)


def build_nc():
    nc = bacc.Bacc()
    io = {}
    io["xq"] = nc.declare_dram_parameter("xq", [L, D], F32, isOutput=False)
    io["xk"] = nc.declare_dram_parameter("xk", [S, D], F32, isOutput=False)
    io["xv"] = nc.declare_dram_parameter("xv", [S, D], F32, isOutput=False)
    io["wq"] = nc.declare_dram_parameter("wq", [128, DC, EG], F16, isOutput=False)
    io["wk"] = nc.declare_dram_parameter("wk", [128, DC, EG], F16, isOutput=False)
    io["wv"] = nc.declare_dram_parameter("wv", [128, DC, EG], F16, isOutput=False)
    io["wo"] = nc.declare_dram_parameter("wo", [64, HG, D], F16, isOutput=False)
    io["bq"] = nc.declare_dram_parameter("bq", [128, 2], F32, isOutput=False)
    io["bk"] = nc.declare_dram_parameter("bk", [128, 2], F32, isOutput=False)
    io["out"] = nc.declare_dram_parameter("out", [L, D], F32, isOutput=True)
    with tile.TileContext(nc) as tc:
        with ExitStack() as ctx:
            _emit(ctx, tc, io)
    nc.compile()
    return nc


_NC = None


def _get_nc():
    global _NC
    if _NC is None:
        _NC = build_nc()
    return _NC


def _chunk_w(w):
    """[512, n] -> [128, 4, n] fp16:  [p, c, :] = w[128c+p, :]"""
    n = w.shape[1]
    return np.ascontiguousarray(
        w.reshape(DC, 128, n).transpose(1, 0, 2), dtype=np.float16)


def make_in_maps(queries, keys, values, tau, Wq, bq, Wk, bk, Wv, bv, Wo):
    in_maps = []
    for c in range(N_CORES):
        b, g = c // 2, c % 2
        e0 = g * EG
        f = np.float32(SCALE * tau[b])
        wq = _chunk_w(Wq[:, e0:e0 + EG] * f)
        wk = _chunk_w(Wk[:, e0:e0 + EG])
        wv = _chunk_w(Wv[:, e0:e0 + EG])
        wo = np.ascontiguousarray(
            Wo[e0:e0 + EG, :].reshape(HG, 64, D).transpose(1, 0, 2),
            dtype=np.float16)
        in_maps.append({
            "xq": np.ascontiguousarray(queries[b], dtype=np.float32),
            "xk": np.ascontiguousarray(keys[b], dtype=np.float32),
            "xv": np.ascontiguousarray(values[b], dtype=np.float32),
            "wq": wq, "wk": wk, "wv": wv, "wo": wo,
            "bq": np.ascontiguousarray(
                (bq[e0:e0 + EG] * f).reshape(2, 128).T, dtype=np.float32),
            "bk": np.ascontiguousarray(
                bk[e0:e0 + EG].reshape(2, 128).T, dtype=np.float32),
        })
    return in_maps


def kernel(queries, keys, values, tau, delta, Wq, bq, Wk, bk, Wv, bv, Wo, bo,
           **_unused):
    queries = np.asarray(queries, dtype=np.float32)
    keys = np.asarray(keys, dtype=np.float32)
    values = np.asarray(values, dtype=np.float32)
    tau = np.asarray(tau, dtype=np.float32)
    Wq, bq = np.asarray(Wq, np.float32), np.asarray(bq, np.float32)
    Wk, bk = np.asarray(Wk, np.float32), np.asarray(bk, np.float32)
    Wv, bv = np.asarray(Wv, np.float32), np.asarray(bv, np.float32)
    Wo, bo = np.asarray(Wo, np.float32), np.asarray(bo, np.float32)

    nc = _get_nc()
    in_maps = make_in_maps(queries, keys, values, tau, Wq, bq, Wk, bk, Wv, bv, Wo)
    res = run_bass_kernel_spmd(nc, in_maps, list(range(N_CORES)))
    # attn rows sum to 1 -> +bv flows through Wo as a constant row; + bo.
    const_row = (bv @ Wo + bo).astype(np.float32)  # [512]
    out = np.empty((B, L, D), dtype=np.float32)
    for b in range(B):
        out[b] = res.results[2 * b]["out"] + res.results[2 * b + 1]["out"] \
            + const_row
    return out


if __name__ == "__main__":
    nc = build_nc()
    print("built OK")


# revision 26
# speedup vs baseline: 1.1842x; 1.1842x over previous
"""DSAttention layer for Trainium2, 8 NeuronCores.

Sharding: core c -> batch b = c//2, head-group g = c%2 (4 heads each,
e-columns 256g..256g+255 of the 512-wide head dim).  tau[b]/8 (softmax
temperature x 1/sqrt(E)) is folded into each core's Wq/bq slice on the
host; delta[b] broadcasts over the softmax axis and is shift-invariant,
so it drops out exactly.  Each core emits its head-group's partial
output projection [2048, 512]; the host sums the pair per batch and
adds (bv @ Wo + bo).

Device dataflow per core (all matmul operands fp16, fp32 PSUM accum):
  X[q|k|v] [2048,512] --PE transpose--> X^T (d-major)
  qT/kT [e 256, l 2048] = W^T @ X^T   (e on partitions)
  v     [s 2048, e 256] -> fp16 v_aug [s,65] per head (ones col -> Z)
  scoresT[s,l] = kT.T @ qT  per head, head pairs concurrent via
                 partition-offset row groups (K=64 at rows 0-63/64-127)
  E = exp(scoresT - 2) fp16  (one ACT instr per [128, 2x512] pair tile)
  attnT_aug[65,l] = v_aug.T @ E  (accumulate 16 s-chunks in PSUM;
                 row 64 = softmax denominator Z)
  normalize: 1/Z (DVE) -> broadcast across partitions via K=1 matmul
                 -> attnT[64, h, l] in SBUF
  out[l,512] = sum_h attnT_h.T @ Wo_h  (K=64, accumulate 4 heads)
"""

import numpy as np
from contextlib import ExitStack

import concourse.bass as bass
import concourse.bacc as bacc
import concourse.mybir as mybir
import concourse.tile as tile
from concourse.bass_utils import run_bass_kernel_spmd
from concourse.masks import make_identity

F32 = mybir.dt.float32
F16 = mybir.dt.float16

B, L, S, D = 4, 2048, 2048, 512
H, E = 8, 64          # full model heads / head dim
HG = 4                # heads per core (head-group)
EG = HG * E           # 256, e-columns per core
N_CORES = 8

LT = L // 128         # 16 l-tiles
ST = S // 128         # 16 s-tiles
DC = D // 128         # 4 d-chunks
LQ = 4                # l-quarters of 512
SCALE = 1.0 / np.sqrt(np.float32(E))
EXP_SHIFT = -2.0      # exp(x-2): cancels in softmax, guards fp16 overflow


def _emit(ctx: ExitStack, tc: "tile.TileContext", io: dict):
    nc = tc.nc
    mm = nc.tensor.matmul

    singles = ctx.enter_context(tc.tile_pool(name="singles", bufs=1))
    bigs = ctx.enter_context(tc.tile_pool(name="bigs", bufs=1))
    xin_pool = ctx.enter_context(tc.tile_pool(name="xin", bufs=2))
    xt_pool = ctx.enter_context(tc.tile_pool(name="xt", bufs=2))
    e_pool = ctx.enter_context(tc.tile_pool(name="eslab", bufs=4))
    z_pool = ctx.enter_context(tc.tile_pool(name="zrec", bufs=2))
    ob_pool = ctx.enter_context(tc.tile_pool(name="outsb", bufs=3))

# One PSUM pool, statically 8 banks: sc 2x2 + avpj 1x2 + small 1x2.
    # "avpj" is reused: projection accumulators (prolog) then AV accumulators
    # (attention).  "small" is reused: transposes (prolog) then zb/wo psums.
    ps = ctx.enter_context(tc.tile_pool(name="ps", bufs=2, space="PSUM"))
    ps3 = ctx.enter_context(tc.tile_pool(name="ps3", bufs=3, space="PSUM"))
    ps1 = ctx.enter_context(tc.tile_pool(name="ps1", bufs=1, space="PSUM"))

    # ---- constants & weights -------------------------------------------
    ident = singles.tile([128, 128], F16)
    make_identity(nc, ident)
    ones_row = singles.tile([1, 128], F16)
    nc.vector.memset(ones_row, 1.0)
    shift_col = singles.tile([128, 1], F32)
    nc.vector.memset(shift_col, EXP_SHIFT)

    wq_sb = singles.tile([128, DC, EG], F16)   # [p, c, e] = Wq[c*128+p, e]
    wk_sb = singles.tile([128, DC, EG], F16)
    wv_sb = singles.tile([128, DC, EG], F16)
    wo_sb = singles.tile([64, HG, D], F16)     # [r, h, n] = Wo[64h+r, n]
    bq_sb = singles.tile([128, 2], F32)        # [p, ec] = bq[128ec+p]
    bk_sb = singles.tile([128, 2], F32)
    nc.sync.dma_start(out=wq_sb, in_=io["wq"][:])
    nc.sync.dma_start(out=wk_sb, in_=io["wk"][:])
    nc.sync.dma_start(out=wv_sb, in_=io["wv"][:])
    nc.sync.dma_start(out=wo_sb, in_=io["wo"][:])
    nc.sync.dma_start(out=bq_sb, in_=io["bq"][:])
    nc.sync.dma_start(out=bk_sb, in_=io["bk"][:])

    # ---- big persistent SBUF tensors -----------------------------------
    qT = bigs.tile([128, 2, L], F16, tag="qT")     # [e_in_chunk, ec, l]
    kT = bigs.tile([128, 2, S], F16, tag="kT")
    v_sb = bigs.tile([128, ST, HG, 65], F16, tag="v")  # [s_in_tile, st, h, dv+1]
    attnT = bigs.tile([64, HG, L], F16, tag="attnT")   # [e_in_head, h, l]
    nc.vector.memset(v_sb, 1.0)  # col 64 stays 1.0 (ones -> Z row)

    # ---- X^T + projections ---------------------------------------------
    def load_transpose(x_dram, lc):
        """DMA X rows [512lc .. 512lc+512) and produce xt[:, c, :] =
        X^T slab [128 d, 4 l-tiles * 128] for this l-chunk."""
        xin = xin_pool.tile([128, 4, D], F32, tag="xin")
        xc = xin_pool.tile([128, 4, D], F16, tag="xc")
        rows = x_dram[lc * 512:(lc + 1) * 512, :].rearrange(
            "(i p) d -> p i d", p=128)
        nc.sync.dma_start(out=xin, in_=rows)
        for i in range(4):
            nc.vector.tensor_copy(out=xc[:, i, :], in_=xin[:, i, :])
        xt = xt_pool.tile([128, DC, 512], F16, tag="xt")
        for c in range(DC):
            tp = ps3.tile([128, 512], F16, tag="avpj", name=f"tp_{lc}_{c}")
            for i in range(4):
                nc.tensor.transpose(tp[:, i * 128:(i + 1) * 128],
                                    xc[:, i, c * 128:(c + 1) * 128], ident)
            nc.vector.tensor_copy(out=xt[:, c, :], in_=tp)
        return xt

    def proj_qk(xt, w_sb, b_sb, dst, lc):
        # dst[:, ec, 512lc : 512lc+512] = (W.T @ X^T) + bias
        for ec in range(2):
            pp = ps3.tile([128, 512], F32, tag="avpj", name=f"pp_{lc}_{ec}")
            for c in range(DC):
                mm(pp, lhsT=w_sb[:, c, ec * 128:(ec + 1) * 128],
                   rhs=xt[:, c, :], start=(c == 0), stop=(c == DC - 1))
            nc.vector.tensor_scalar_add(
                out=dst[:, ec, lc * 512:(lc + 1) * 512], in0=pp,
                scalar1=b_sb[:, ec:ec + 1])

    def proj_v(xt, lc):
        for i in range(4):
            st = lc * 4 + i
            vp = ps3.tile([128, 512], F32, tag="avpj", name=f"vp_{lc}_{i}")[:, 0:EG]
            for c in range(DC):
                mm(vp, lhsT=xt[:, c, i * 128:(i + 1) * 128],
                   rhs=wv_sb[:, c, :], start=(c == 0), stop=(c == DC - 1))
            for h in range(HG):
                nc.vector.tensor_copy(out=v_sb[:, st, h, 0:64],
                                      in_=vp[:, h * 64:(h + 1) * 64])

    for lc in range(4):
        xt = load_transpose(io["xk"], lc)
        proj_qk(xt, wk_sb, bk_sb, kT, lc)
    for lc in range(4):
        xt = load_transpose(io["xv"], lc)
        proj_v(xt, lc)

    # ---- attention (q projection interleaved per l-quarter) -------------
    for lq in range(LQ):
        l0 = lq * 512
        xt = load_transpose(io["xq"], lq)
        proj_qk(xt, wq_sb, bq_sb, qT, lq)
        for p in range(2):                      # head pair
            av = [ps3.tile([65, 512], F32, tag="avpj", name=f"av{lq}_{p}_{i}")
                  for i in range(2)]
            for j in range(ST):
                sc = ps.tile([128, 2, 512], F32, tag="sc", name=f"sc_{lq}_{p}_{j}")
                ep = e_pool.tile([128, 2, 512], F16, tag="ep")
                for hh in range(2):             # rows 0-63 / 64-127: concurrent
                    o = hh * 64
                    mm(sc[:, hh, :],
                       lhsT=kT[o:o + 64, p, j * 128:(j + 1) * 128],
                       rhs=qT[o:o + 64, p, l0:l0 + 512],
                       start=True, stop=True)
                nc.scalar.activation(out=ep, in_=sc,
                                     func=mybir.ActivationFunctionType.Exp,
                                     bias=shift_col[:, 0:1], scale=1.0)
                for hh in range(2):
                    mm(av[hh], lhsT=v_sb[:, j, 2 * p + hh, :],
                       rhs=ep[:, hh, :], start=(j == 0), stop=(j == ST - 1))
            for hh in range(2):
                h = 2 * p + hh
                zrow = z_pool.tile([1, 512], F16, tag="zrow")
                nc.vector.tensor_copy(out=zrow, in_=av[hh][64:65, :])
                zcol = ps1.tile([128, 8], F16, tag="small", name=f"zc{lq}{p}{hh}")
                for c in range(4):
                    nc.tensor.transpose(zcol[:, 2 * c:2 * c + 1],
                                        zrow[0:1, c * 128:(c + 1) * 128],
                                        ident[0:1, 0:1])
                rcol = z_pool.tile([128, 4], F16, tag="rcol")
                with nc.allow_low_precision(reason="1/Z row scale in f16"):
                    nc.vector.reciprocal(rcol, zcol[:, 0:8:2])
                rrow = ps1.tile([1, 512], F16, tag="small", name=f"rr{lq}{p}{hh}")
                for c in range(4):
                    nc.tensor.transpose(rrow[0:1, c * 128:(c + 1) * 128],
                                        rcol[:, c:c + 1], ident)
                rrow_sb = z_pool.tile([1, 512], F16, tag="rrow_sb")
                nc.vector.tensor_copy(out=rrow_sb, in_=rrow)
                zb = ps1.tile([64, 512], F32, tag="small", name=f"zb{lq}{p}{hh}")
                mm(zb, lhsT=ones_row[0:1, 0:64], rhs=rrow_sb,
                   start=True, stop=True)
                zb_sb = z_pool.tile([64, 512], F32, tag="zb_sb")
                nc.vector.tensor_copy(out=zb_sb, in_=zb)
                nc.vector.tensor_mul(out=attnT[:, h, l0:l0 + 512],
                                     in0=av[hh][0:64, :], in1=zb_sb)
        # output projection for this l-quarter
        for i in range(4):
            lt = lq * 4 + i
            op = ps3.tile([128, D], F32, tag="avpj", name=f"op_{lq}_{i}")
            for h in range(HG):
                mm(op, lhsT=attnT[:, h, lt * 128:(lt + 1) * 128],
                   rhs=wo_sb[:, h, :], start=(h == 0), stop=(h == HG - 1))
            ob = ob_pool.tile([128, D], F32, tag="ob")
            nc.vector.tensor_copy(out=ob, in_=op)
            nc.sync.dma_start(out=io["out"][lt * 128:(lt + 1) * 128, :], in_=ob)


def build_nc():
    nc = bacc.Bacc()
    io = {}
    io["xq"] = nc.declare_dram_parameter("xq", [L, D], F32, isOutput=False)
    io["xk"] = nc.declare_dram_parameter("xk", [S, D], F32, isOutput=False)
    io["xv"] = nc.declare_dram_parameter("xv", [S, D], F32, isOutput=False)
    io["wq"] = nc.declare_dram_parameter("wq", [128, DC, EG], F16, isOutput=False)
    io["wk"] = nc.declare_dram_parameter("wk", [128, DC, EG], F16, isOutput=False)
    io["wv"] = nc.declare_dram_parameter("wv", [128, DC, EG], F16, isOutput=False)
    io["wo"] = nc.declare_dram_parameter("wo", [64, HG, D], F16, isOutput=False)
    io["bq"] = nc.declare_dram_parameter("bq", [128, 2], F32, isOutput=False)
    io["bk"] = nc.declare_dram_parameter("bk", [128, 2], F32, isOutput=False)
    io["out"] = nc.declare_dram_parameter("out", [L, D], F32, isOutput=True)
    with tile.TileContext(nc) as tc:
        with ExitStack() as ctx:
            _emit(ctx, tc, io)
    nc.compile()
    return nc


_NC = None


def _get_nc():
    global _NC
    if _NC is None:
        _NC = build_nc()
    return _NC


def _chunk_w(w):
    """[512, n] -> [128, 4, n] fp16:  [p, c, :] = w[128c+p, :]"""
    n = w.shape[1]
    return np.ascontiguousarray(
        w.reshape(DC, 128, n).transpose(1, 0, 2), dtype=np.float16)


def make_in_maps(queries, keys, values, tau, Wq, bq, Wk, bk, Wv, bv, Wo):
    in_maps = []
    for c in range(N_CORES):
        b, g = c // 2, c % 2
        e0 = g * EG
        f = np.float32(SCALE * tau[b])
        wq = _chunk_w(Wq[:, e0:e0 + EG] * f)
        wk = _chunk_w(Wk[:, e0:e0 + EG])
        wv = _chunk_w(Wv[:, e0:e0 + EG])
        wo = np.ascontiguousarray(
            Wo[e0:e0 + EG, :].reshape(HG, 64, D).transpose(1, 0, 2),
            dtype=np.float16)
        in_maps.append({
            "xq": np.ascontiguousarray(queries[b], dtype=np.float32),
            "xk": np.ascontiguousarray(keys[b], dtype=np.float32),
            "xv": np.ascontiguousarray(values[b], dtype=np.float32),
            "wq": wq, "wk": wk, "wv": wv, "wo": wo,
            "bq": np.ascontiguousarray(
                (bq[e0:e0 + EG] * f).reshape(2, 128).T, dtype=np.float32),
            "bk": np.ascontiguousarray(
                bk[e0:e0 + EG].reshape(2, 128).T, dtype=np.float32),
        })
    return in_maps


def kernel(queries, keys, values, tau, delta, Wq, bq, Wk, bk, Wv, bv, Wo, bo,
           **_unused):
    queries = np.asarray(queries, dtype=np.float32)
    keys = np.asarray(keys, dtype=np.float32)
    values = np.asarray(values, dtype=np.float32)
    tau = np.asarray(tau, dtype=np.float32)
    Wq, bq = np.asarray(Wq, np.float32), np.asarray(bq, np.float32)
    Wk, bk = np.asarray(Wk, np.float32), np.asarray(bk, np.float32)
    Wv, bv = np.asarray(Wv, np.float32), np.asarray(bv, np.float32)
    Wo, bo = np.asarray(Wo, np.float32), np.asarray(bo, np.float32)

    nc = _get_nc()
    in_maps = make_in_maps(queries, keys, values, tau, Wq, bq, Wk, bk, Wv, bv, Wo)
    res = run_bass_kernel_spmd(nc, in_maps, list(range(N_CORES)))
    # attn rows sum to 1 -> +bv flows through Wo as a constant row; + bo.
    const_row = (bv @ Wo + bo).astype(np.float32)  # [512]
    out = np.empty((B, L, D), dtype=np.float32)
    for b in range(B):
        out[b] = res.results[2 * b]["out"] + res.results[2 * b + 1]["out"] \
            + const_row
    return out


if __name__ == "__main__":
    nc = build_nc()
    print("built OK")


# revision 27
# speedup vs baseline: 1.2571x; 1.0616x over previous
"""DSAttention layer for Trainium2, 8 NeuronCores.

Sharding: core c -> batch b = c//2, head-group g = c%2 (4 heads each,
e-columns 256g..256g+255 of the 512-wide head dim).  tau[b]/8 (softmax
temperature x 1/sqrt(E)) is folded into each core's Wq/bq slice on the
host; delta[b] broadcasts over the softmax axis and is shift-invariant,
so it drops out exactly.  Each core emits its head-group's partial
output projection [2048, 512]; the host sums the pair per batch and
adds (bv @ Wo + bo).

Device dataflow per core (all matmul operands fp16, fp32 PSUM accum):
  X[q|k|v] [2048,512] --PE transpose--> X^T (d-major)
  qT/kT [e 256, l 2048] = W^T @ X^T   (e on partitions)
  v     [s 2048, e 256] -> fp16 v_aug [s,65] per head (ones col -> Z)
  scoresT[s,l] = kT.T @ qT  per head, head pairs concurrent via
                 partition-offset row groups (K=64 at rows 0-63/64-127)
  E = exp(scoresT - 2) fp16  (one ACT instr per [128, 2x512] pair tile)
  attnT_aug[65,l] = v_aug.T @ E  (accumulate 16 s-chunks in PSUM;
                 row 64 = softmax denominator Z)
  normalize: 1/Z (DVE) -> broadcast across partitions via K=1 matmul
                 -> attnT[64, h, l] in SBUF
  out[l,512] = sum_h attnT_h.T @ Wo_h  (K=64, accumulate 4 heads)
"""

import numpy as np
from contextlib import ExitStack

import concourse.bass as bass
import concourse.bacc as bacc
import concourse.mybir as mybir
import concourse.tile as tile
from concourse.bass_utils import run_bass_kernel_spmd
from concourse.masks import make_identity

F32 = mybir.dt.float32
F16 = mybir.dt.float16

B, L, S, D = 4, 2048, 2048, 512
H, E = 8, 64          # full model heads / head dim
HG = 4                # heads per core (head-group)
EG = HG * E           # 256, e-columns per core
N_CORES = 8

LT = L // 128         # 16 l-tiles
ST = S // 128         # 16 s-tiles
DC = D // 128         # 4 d-chunks
LQ = 4                # l-quarters of 512
SCALE = 1.0 / np.sqrt(np.float32(E))
EXP_SHIFT = -2.0      # exp(x-2): cancels in softmax, guards fp16 overflow


def _emit(ctx: ExitStack, tc: "tile.TileContext", io: dict):
    nc = tc.nc
    mm = nc.tensor.matmul

    singles = ctx.enter_context(tc.tile_pool(name="singles", bufs=1))
    bigs = ctx.enter_context(tc.tile_pool(name="bigs", bufs=1))
    xin_pool = ctx.enter_context(tc.tile_pool(name="xin", bufs=2))
    xt_pool = ctx.enter_context(tc.tile_pool(name="xt", bufs=2))
    e_pool = ctx.enter_context(tc.tile_pool(name="eslab", bufs=4))
    z_pool = ctx.enter_context(tc.tile_pool(name="zrec", bufs=2))
    ob_pool = ctx.enter_context(tc.tile_pool(name="outsb", bufs=3))

# One PSUM pool, statically 8 banks: sc 2x2 + avpj 1x2 + small 1x2.
    # "avpj" is reused: projection accumulators (prolog) then AV accumulators
    # (attention).  "small" is reused: transposes (prolog) then zb/wo psums.
    ps = ctx.enter_context(tc.tile_pool(name="ps", bufs=2, space="PSUM"))
    ps3 = ctx.enter_context(tc.tile_pool(name="ps3", bufs=3, space="PSUM"))
    ps1 = ctx.enter_context(tc.tile_pool(name="ps1", bufs=1, space="PSUM"))

    # ---- constants & weights -------------------------------------------
    ident = singles.tile([128, 128], F16)
    make_identity(nc, ident)
    ones_row = singles.tile([1, 128], F16)
    nc.vector.memset(ones_row, 1.0)
    shift_col = singles.tile([128, 1], F32)
    nc.vector.memset(shift_col, EXP_SHIFT)

    wq_sb = singles.tile([128, DC, EG], F16)   # [p, c, e] = Wq[c*128+p, e]
    wk_sb = singles.tile([128, DC, EG], F16)
    wv_sb = singles.tile([128, DC, EG], F16)
    wo_sb = singles.tile([64, HG, D], F16)     # [r, h, n] = Wo[64h+r, n]
    bq_sb = singles.tile([128, 2], F32)        # [p, ec] = bq[128ec+p]
    bk_sb = singles.tile([128, 2], F32)
    nc.sync.dma_start(out=wq_sb, in_=io["wq"][:])
    nc.sync.dma_start(out=wk_sb, in_=io["wk"][:])
    nc.sync.dma_start(out=wv_sb, in_=io["wv"][:])
    nc.sync.dma_start(out=wo_sb, in_=io["wo"][:])
    nc.sync.dma_start(out=bq_sb, in_=io["bq"][:])
    nc.sync.dma_start(out=bk_sb, in_=io["bk"][:])

    # ---- big persistent SBUF tensors -----------------------------------
    qT = bigs.tile([128, 2, L], F16, tag="qT")     # [e_in_chunk, ec, l]
    kT = bigs.tile([128, 2, S], F16, tag="kT")
    v_sb = bigs.tile([128, ST, HG, 65], F16, tag="v")  # [s_in_tile, st, h, dv+1]
    attnT = bigs.tile([64, HG, L], F16, tag="attnT")   # [e_in_head, h, l]
    nc.vector.memset(v_sb[:, :, :, 64:65], 1.0)  # ones col -> Z row

    # ---- X^T + projections ---------------------------------------------
    def load_transpose(x_dram, lc):
        """DMA X rows [512lc .. 512lc+512) and produce xt[:, c, :] =
        X^T slab [128 d, 4 l-tiles * 128] for this l-chunk."""
        xin = xin_pool.tile([128, 4, D], F32, tag="xin")
        xc = xin_pool.tile([128, 4, D], F16, tag="xc")
        rows = x_dram[lc * 512:(lc + 1) * 512, :].rearrange(
            "(i p) d -> p i d", p=128)
        nc.sync.dma_start(out=xin, in_=rows)
        for i in range(4):
            nc.vector.tensor_copy(out=xc[:, i, :], in_=xin[:, i, :])
        xt = xt_pool.tile([128, DC, 512], F16, tag="xt")
        for c in range(DC):
            tp = ps3.tile([128, 512], F16, tag="avpj", name=f"tp_{lc}_{c}")
            for i in range(4):
                nc.tensor.transpose(tp[:, i * 128:(i + 1) * 128],
                                    xc[:, i, c * 128:(c + 1) * 128], ident)
            nc.vector.tensor_copy(out=xt[:, c, :], in_=tp)
        return xt

    def proj_qk(xt, w_sb, b_sb, dst, lc):
        # dst[:, ec, 512lc : 512lc+512] = (W.T @ X^T) + bias
        for ec in range(2):
            pp = ps3.tile([128, 512], F32, tag="avpj", name=f"pp_{lc}_{ec}")
            for c in range(DC):
                mm(pp, lhsT=w_sb[:, c, ec * 128:(ec + 1) * 128],
                   rhs=xt[:, c, :], start=(c == 0), stop=(c == DC - 1))
            nc.vector.tensor_scalar_add(
                out=dst[:, ec, lc * 512:(lc + 1) * 512], in0=pp,
                scalar1=b_sb[:, ec:ec + 1])

    def proj_v(xt, lc):
        for i in range(4):
            st = lc * 4 + i
            vp = ps3.tile([128, 512], F32, tag="avpj", name=f"vp_{lc}_{i}")[:, 0:EG]
            for c in range(DC):
                mm(vp, lhsT=xt[:, c, i * 128:(i + 1) * 128],
                   rhs=wv_sb[:, c, :], start=(c == 0), stop=(c == DC - 1))
            nc.vector.tensor_copy(
                out=v_sb[:, st, :, 0:64],
                in_=vp.rearrange("p (h e) -> p h e", h=HG))

    for lc in range(4):
        xt = load_transpose(io["xk"], lc)
        proj_qk(xt, wk_sb, bk_sb, kT, lc)
    for lc in range(4):
        xt = load_transpose(io["xv"], lc)
        proj_v(xt, lc)

    # ---- attention (q projection prefetched one l-quarter ahead) --------
    xt = load_transpose(io["xq"], 0)
    proj_qk(xt, wq_sb, bq_sb, qT, 0)
    for lq in range(LQ):
        l0 = lq * 512
        if lq + 1 < LQ:
            xt = load_transpose(io["xq"], lq + 1)
            proj_qk(xt, wq_sb, bq_sb, qT, lq + 1)
        for p in range(2):                      # head pair
            av = [ps3.tile([65, 512], F32, tag="avpj", name=f"av{lq}_{p}_{i}")
                  for i in range(2)]
            for j in range(ST):
                sc = ps.tile([128, 2, 512], F32, tag="sc", name=f"sc_{lq}_{p}_{j}")
                ep = e_pool.tile([128, 2, 512], F16, tag="ep")
                for hh in range(2):             # rows 0-63 / 64-127: concurrent
                    o = hh * 64
                    mm(sc[:, hh, :],
                       lhsT=kT[o:o + 64, p, j * 128:(j + 1) * 128],
                       rhs=qT[o:o + 64, p, l0:l0 + 512],
                       start=True, stop=True, tile_position=(o, 0))
                nc.scalar.activation(out=ep, in_=sc,
                                     func=mybir.ActivationFunctionType.Exp,
                                     bias=shift_col[:, 0:1], scale=1.0)
                for hh in range(2):
                    mm(av[hh], lhsT=v_sb[:, j, 2 * p + hh, :],
                       rhs=ep[:, hh, :], start=(j == 0), stop=(j == ST - 1))
            for hh in range(2):
                h = 2 * p + hh
                zrow = z_pool.tile([1, 512], F16, tag="zrow")
                nc.vector.tensor_copy(out=zrow, in_=av[hh][64:65, :])
                zcol = ps1.tile([128, 8], F16, tag="small", name=f"zc{lq}{p}{hh}")
                for c in range(4):
                    nc.tensor.transpose(zcol[:, 2 * c:2 * c + 1],
                                        zrow[0:1, c * 128:(c + 1) * 128],
                                        ident[0:1, 0:1])
                rcol = z_pool.tile([128, 4], F16, tag="rcol")
                with nc.allow_low_precision(reason="1/Z row scale in f16"):
                    nc.vector.reciprocal(rcol, zcol[:, 0:8:2])
                rrow = ps1.tile([1, 512], F16, tag="small", name=f"rr{lq}{p}{hh}")
                for c in range(4):
                    nc.tensor.transpose(rrow[0:1, c * 128:(c + 1) * 128],
                                        rcol[:, c:c + 1], ident)
                rrow_sb = z_pool.tile([1, 512], F16, tag="rrow_sb")
                nc.vector.tensor_copy(out=rrow_sb, in_=rrow)
                zb = ps1.tile([64, 512], F32, tag="small", name=f"zb{lq}{p}{hh}")
                mm(zb, lhsT=ones_row[0:1, 0:64], rhs=rrow_sb,
                   start=True, stop=True)
                zb_sb = z_pool.tile([64, 512], F32, tag="zb_sb")
                nc.vector.tensor_copy(out=zb_sb, in_=zb)
                nc.vector.tensor_mul(out=attnT[:, h, l0:l0 + 512],
                                     in0=av[hh][0:64, :], in1=zb_sb)
        # output projection for this l-quarter
        for i in range(4):
            lt = lq * 4 + i
            op = ps3.tile([128, D], F32, tag="avpj", name=f"op_{lq}_{i}")
            for h in range(HG):
                mm(op, lhsT=attnT[:, h, lt * 128:(lt + 1) * 128],
                   rhs=wo_sb[:, h, :], start=(h == 0), stop=(h == HG - 1))
            ob = ob_pool.tile([128, D], F32, tag="ob")
            nc.vector.tensor_copy(out=ob, in_=op)
            nc.sync.dma_start(out=io["out"][lt * 128:(lt + 1) * 128, :], in_=ob)


def build_nc():
    nc = bacc.Bacc()
    io = {}
    io["xq"] = nc.declare_dram_parameter("xq", [L, D], F32, isOutput=False)
    io["xk"] = nc.declare_dram_parameter("xk", [S, D], F32, isOutput=False)
    io["xv"] = nc.declare_dram_parameter("xv", [S, D], F32, isOutput=False)
    io["wq"] = nc.declare_dram_parameter("wq", [128, DC, EG], F16, isOutput=False)
    io["wk"] = nc.declare_dram_parameter("wk", [128, DC, EG], F16, isOutput=False)
    io["wv"] = nc.declare_dram_parameter("wv", [128, DC, EG], F16, isOutput=False)
    io["wo"] = nc.declare_dram_parameter("wo", [64, HG, D], F16, isOutput=False)
    io["bq"] = nc.declare_dram_parameter("bq", [128, 2], F32, isOutput=False)
    io["bk"] = nc.declare_dram_parameter("bk", [128, 2], F32, isOutput=False)
    io["out"] = nc.declare_dram_parameter("out", [L, D], F32, isOutput=True)
    with tile.TileContext(nc) as tc:
        with ExitStack() as ctx:
            _emit(ctx, tc, io)
    nc.compile()
    return nc


_NC = None


def _get_nc():
    global _NC
    if _NC is None:
        _NC = build_nc()
    return _NC


def _chunk_w(w):
    """[512, n] -> [128, 4, n] fp16:  [p, c, :] = w[128c+p, :]"""
    n = w.shape[1]
    return np.ascontiguousarray(
        w.reshape(DC, 128, n).transpose(1, 0, 2), dtype=np.float16)


def make_in_maps(queries, keys, values, tau, Wq, bq, Wk, bk, Wv, bv, Wo):
    in_maps = []
    for c in range(N_CORES):
        b, g = c // 2, c % 2
        e0 = g * EG
        f = np.float32(SCALE * tau[b])
        wq = _chunk_w(Wq[:, e0:e0 + EG] * f)
        wk = _chunk_w(Wk[:, e0:e0 + EG])
        wv = _chunk_w(Wv[:, e0:e0 + EG])
        wo = np.ascontiguousarray(
            Wo[e0:e0 + EG, :].reshape(HG, 64, D).transpose(1, 0, 2),
            dtype=np.float16)
        in_maps.append({
            "xq": np.ascontiguousarray(queries[b], dtype=np.float32),
            "xk": np.ascontiguousarray(keys[b], dtype=np.float32),
            "xv": np.ascontiguousarray(values[b], dtype=np.float32),
            "wq": wq, "wk": wk, "wv": wv, "wo": wo,
            "bq": np.ascontiguousarray(
                (bq[e0:e0 + EG] * f).reshape(2, 128).T, dtype=np.float32),
            "bk": np.ascontiguousarray(
                bk[e0:e0 + EG].reshape(2, 128).T, dtype=np.float32),
        })
    return in_maps


def kernel(queries, keys, values, tau, delta, Wq, bq, Wk, bk, Wv, bv, Wo, bo,
           **_unused):
    queries = np.asarray(queries, dtype=np.float32)
    keys = np.asarray(keys, dtype=np.float32)
    values = np.asarray(values, dtype=np.float32)
    tau = np.asarray(tau, dtype=np.float32)
    Wq, bq = np.asarray(Wq, np.float32), np.asarray(bq, np.float32)
    Wk, bk = np.asarray(Wk, np.float32), np.asarray(bk, np.float32)
    Wv, bv = np.asarray(Wv, np.float32), np.asarray(bv, np.float32)
    Wo, bo = np.asarray(Wo, np.float32), np.asarray(bo, np.float32)

    nc = _get_nc()
    in_maps = make_in_maps(queries, keys, values, tau, Wq, bq, Wk, bk, Wv, bv, Wo)
    res = run_bass_kernel_spmd(nc, in_maps, list(range(N_CORES)))
    # attn rows sum to 1 -> +bv flows through Wo as a constant row; + bo.
    const_row = (bv @ Wo + bo).astype(np.float32)  # [512]
    out = np.empty((B, L, D), dtype=np.float32)
    for b in range(B):
        out[b] = res.results[2 * b]["out"] + res.results[2 * b + 1]["out"] \
            + const_row
    return out


if __name__ == "__main__":
    nc = build_nc()
    print("built OK")


# revision 28
# speedup vs baseline: 1.3452x; 1.0700x over previous
"""DSAttention layer for Trainium2, 8 NeuronCores.

Sharding: core c -> batch b = c//2, head-group g = c%2 (4 heads each,
e-columns 256g..256g+255 of the 512-wide head dim).  tau[b]/8 (softmax
temperature x 1/sqrt(E)) is folded into each core's Wq/bq slice on the
host; delta[b] broadcasts over the softmax axis and is shift-invariant,
so it drops out exactly.  Each core emits its head-group's partial
output projection [2048, 512]; the host sums the pair per batch and
adds (bv @ Wo + bo).

Device dataflow per core (all matmul operands fp16, fp32 PSUM accum):
  X[q|k|v] [2048,512] --PE transpose--> X^T (d-major)
  qT/kT [e 256, l 2048] = W^T @ X^T   (e on partitions)
  v     [s 2048, e 256] -> fp16 v_aug [s,65] per head (ones col -> Z)
  scoresT[s,l] = kT.T @ qT  per head, head pairs concurrent via
                 partition-offset row groups (K=64 at rows 0-63/64-127)
  E = exp(scoresT - 2) fp16  (one ACT instr per [128, 2x512] pair tile)
  attnT_aug[65,l] = v_aug.T @ E  (accumulate 16 s-chunks in PSUM;
                 row 64 = softmax denominator Z)
  normalize: 1/Z (DVE) -> broadcast across partitions via K=1 matmul
                 -> attnT[64, h, l] in SBUF
  out[l,512] = sum_h attnT_h.T @ Wo_h  (K=64, accumulate 4 heads)
"""

import numpy as np
from contextlib import ExitStack

import concourse.bass as bass
import concourse.bacc as bacc
import concourse.mybir as mybir
import concourse.tile as tile
from concourse.bass_utils import run_bass_kernel_spmd
from concourse.masks import make_identity

F32 = mybir.dt.float32
F16 = mybir.dt.float16

B, L, S, D = 4, 2048, 2048, 512
H, E = 8, 64          # full model heads / head dim
HG = 4                # heads per core (head-group)
EG = HG * E           # 256, e-columns per core
N_CORES = 8

LT = L // 128         # 16 l-tiles
ST = S // 128         # 16 s-tiles
DC = D // 128         # 4 d-chunks
LQ = 4                # l-quarters of 512
SCALE = 1.0 / np.sqrt(np.float32(E))
EXP_SHIFT = -2.0      # exp(x-2): cancels in softmax, guards fp16 overflow


def _emit(ctx: ExitStack, tc: "tile.TileContext", io: dict):
    nc = tc.nc
    mm = nc.tensor.matmul

    singles = ctx.enter_context(tc.tile_pool(name="singles", bufs=1))
    bigs = ctx.enter_context(tc.tile_pool(name="bigs", bufs=1))
    xin_pool = ctx.enter_context(tc.tile_pool(name="xin", bufs=2))
    xt_pool = ctx.enter_context(tc.tile_pool(name="xt", bufs=2))
    e_pool = ctx.enter_context(tc.tile_pool(name="eslab", bufs=4))
    z_pool = ctx.enter_context(tc.tile_pool(name="zrec", bufs=2))
    ob_pool = ctx.enter_context(tc.tile_pool(name="outsb", bufs=3))

# One PSUM pool, statically 8 banks: sc 2x2 + avpj 1x2 + small 1x2.
    # "avpj" is reused: projection accumulators (prolog) then AV accumulators
    # (attention).  "small" is reused: transposes (prolog) then zb/wo psums.
    ps = ctx.enter_context(tc.tile_pool(name="ps", bufs=2, space="PSUM"))
    ps_av = ctx.enter_context(tc.tile_pool(name="ps_av", bufs=2, space="PSUM"))
    ps_wk = ctx.enter_context(tc.tile_pool(name="ps_wk", bufs=2, space="PSUM"))

    # ---- constants & weights -------------------------------------------
    ident = singles.tile([128, 128], F16)
    make_identity(nc, ident)
    ones_row = singles.tile([1, 128], F16)
    nc.vector.memset(ones_row, 1.0)
    shift_col = singles.tile([128, 1], F32)
    nc.vector.memset(shift_col, EXP_SHIFT)

    wq_sb = singles.tile([128, DC, EG], F16)   # [p, c, e] = Wq[c*128+p, e]
    wk_sb = singles.tile([128, DC, EG], F16)
    wv_sb = singles.tile([128, DC, EG], F16)
    wo_sb = singles.tile([64, HG, D], F16)     # [r, h, n] = Wo[64h+r, n]
    bq_sb = singles.tile([128, 2], F32)        # [p, ec] = bq[128ec+p]
    bk_sb = singles.tile([128, 2], F32)
    nc.sync.dma_start(out=wq_sb, in_=io["wq"][:])
    nc.sync.dma_start(out=wk_sb, in_=io["wk"][:])
    nc.sync.dma_start(out=wv_sb, in_=io["wv"][:])
    nc.sync.dma_start(out=wo_sb, in_=io["wo"][:])
    nc.sync.dma_start(out=bq_sb, in_=io["bq"][:])
    nc.sync.dma_start(out=bk_sb, in_=io["bk"][:])

    # ---- big persistent SBUF tensors -----------------------------------
    qT = bigs.tile([128, 2, L], F16, tag="qT")     # [e_in_chunk, ec, l]
    kT = bigs.tile([128, 2, S], F16, tag="kT")
    v_sb = bigs.tile([128, ST, HG, 65], F16, tag="v")  # [s_in_tile, st, h, dv+1]
    attnT = bigs.tile([64, HG, L], F16, tag="attnT")   # [e_in_head, h, l]
    nc.vector.memset(v_sb[:, :, :, 64:65], 1.0)  # ones col -> Z row

    # ---- X^T + projections ---------------------------------------------
    def load_transpose(x_dram, lc):
        """DMA X rows [512lc .. 512lc+512) and produce xt[:, c, :] =
        X^T slab [128 d, 4 l-tiles * 128] for this l-chunk."""
        xin = xin_pool.tile([128, 4, D], F32, tag="xin")
        xc = xin_pool.tile([128, 4, D], F16, tag="xc")
        for i in range(4):
            r0 = lc * 512 + i * 128
            nc.sync.dma_start(out=xin[:, i, :], in_=x_dram[r0:r0 + 128, :])
            nc.vector.tensor_copy(out=xc[:, i, :], in_=xin[:, i, :])
        xt = xt_pool.tile([128, DC, 512], F16, tag="xt")
        for c in range(DC):
            tp = ps_wk.tile([128, 512], F16, tag="work", name=f"tp_{lc}_{c}")
            for i in range(4):
                nc.tensor.transpose(tp[:, i * 128:(i + 1) * 128],
                                    xc[:, i, c * 128:(c + 1) * 128], ident)
            nc.vector.tensor_copy(out=xt[:, c, :], in_=tp)
        return xt

    def proj_qk(xt, w_sb, b_sb, dst, lc):
        # dst[:, ec, 512lc : 512lc+512] = (W.T @ X^T) + bias
        for ec in range(2):
            pp = ps_wk.tile([128, 512], F32, tag="work", name=f"pp_{lc}_{ec}")
            for c in range(DC):
                mm(pp, lhsT=w_sb[:, c, ec * 128:(ec + 1) * 128],
                   rhs=xt[:, c, :], start=(c == 0), stop=(c == DC - 1))
            nc.vector.tensor_scalar_add(
                out=dst[:, ec, lc * 512:(lc + 1) * 512], in0=pp,
                scalar1=b_sb[:, ec:ec + 1])

    def proj_v(xt, lc):
        for i in range(4):
            st = lc * 4 + i
            vp = ps_wk.tile([128, 512], F32, tag="work", name=f"vp_{lc}_{i}")[:, 0:EG]
            for c in range(DC):
                mm(vp, lhsT=xt[:, c, i * 128:(i + 1) * 128],
                   rhs=wv_sb[:, c, :], start=(c == 0), stop=(c == DC - 1))
            nc.vector.tensor_copy(
                out=v_sb[:, st, :, 0:64],
                in_=vp.rearrange("p (h e) -> p h e", h=HG))

    xt = load_transpose(io["xk"], 0)
    proj_qk(xt, wk_sb, bk_sb, kT, 0)
    xt = load_transpose(io["xq"], 0)
    proj_qk(xt, wq_sb, bq_sb, qT, 0)
    xt = load_transpose(io["xv"], 0)
    proj_v(xt, 0)
    for lc in range(1, 4):
        xt = load_transpose(io["xk"], lc)
        proj_qk(xt, wk_sb, bk_sb, kT, lc)
        xt = load_transpose(io["xv"], lc)
        proj_v(xt, lc)

    # ---- attention (q projection prefetched one l-quarter ahead) --------
    for lq in range(LQ):
        l0 = lq * 512
        if lq + 1 < LQ:
            xt = load_transpose(io["xq"], lq + 1)
            proj_qk(xt, wq_sb, bq_sb, qT, lq + 1)
        for p in range(2):                      # head pair
            av = [ps_av.tile([65, 512], F32, tag="av", name=f"av{lq}_{p}_{i}")
                  for i in range(2)]
            for j in range(ST):
                sc = ps.tile([128, 2, 512], F32, tag="sc", name=f"sc_{lq}_{p}_{j}")
                ep = e_pool.tile([128, 2, 512], F16, tag="ep")
                for hh in range(2):             # rows 0-63 / 64-127: concurrent
                    o = hh * 64
                    mm(sc[:, hh, :],
                       lhsT=kT[o:o + 64, p, j * 128:(j + 1) * 128],
                       rhs=qT[o:o + 64, p, l0:l0 + 512],
                       start=True, stop=True, tile_position=(o, 0))
                nc.scalar.activation(out=ep, in_=sc,
                                     func=mybir.ActivationFunctionType.Exp,
                                     bias=shift_col[:, 0:1], scale=1.0)
                for hh in range(2):
                    mm(av[hh], lhsT=v_sb[:, j, 2 * p + hh, :],
                       rhs=ep[:, hh, :], start=(j == 0), stop=(j == ST - 1))
            for hh in range(2):
                h = 2 * p + hh
                zrow = z_pool.tile([1, 512], F16, tag="zrow")
                nc.vector.tensor_copy(out=zrow, in_=av[hh][64:65, :])
                zcol = ps_wk.tile([128, 8], F16, tag="work", name=f"zc{lq}{p}{hh}")
                for c in range(4):
                    nc.tensor.transpose(zcol[:, 2 * c:2 * c + 1],
                                        zrow[0:1, c * 128:(c + 1) * 128],
                                        ident[0:1, 0:1])
                rcol = z_pool.tile([128, 4], F16, tag="rcol")
                with nc.allow_low_precision(reason="1/Z row scale in f16"):
                    nc.vector.reciprocal(rcol, zcol[:, 0:8:2])
                rrow = ps_wk.tile([1, 512], F16, tag="work", name=f"rr{lq}{p}{hh}")
                for c in range(4):
                    nc.tensor.transpose(rrow[0:1, c * 128:(c + 1) * 128],
                                        rcol[:, c:c + 1], ident)
                rrow_sb = z_pool.tile([1, 512], F16, tag="rrow_sb")
                nc.vector.tensor_copy(out=rrow_sb, in_=rrow)
                zb = ps_wk.tile([64, 512], F32, tag="work", name=f"zb{lq}{p}{hh}")
                mm(zb, lhsT=ones_row[0:1, 0:64], rhs=rrow_sb,
                   start=True, stop=True)
                zb_sb = z_pool.tile([64, 512], F32, tag="zb_sb")
                nc.vector.tensor_copy(out=zb_sb, in_=zb)
                nc.vector.tensor_mul(out=attnT[:, h, l0:l0 + 512],
                                     in0=av[hh][0:64, :], in1=zb_sb)
        # output projection for this l-quarter
        for i in range(4):
            lt = lq * 4 + i
            op = ps_av.tile([128, D], F32, tag="av", name=f"op_{lq}_{i}")
            for h in range(HG):
                mm(op, lhsT=attnT[:, h, lt * 128:(lt + 1) * 128],
                   rhs=wo_sb[:, h, :], start=(h == 0), stop=(h == HG - 1))
            ob = ob_pool.tile([128, D], F32, tag="ob")
            nc.vector.tensor_copy(out=ob, in_=op)
            nc.sync.dma_start(out=io["out"][lt * 128:(lt + 1) * 128, :], in_=ob)


def build_nc():
    nc = bacc.Bacc()
    io = {}
    io["xq"] = nc.declare_dram_parameter("xq", [L, D], F32, isOutput=False)
    io["xk"] = nc.declare_dram_parameter("xk", [S, D], F32, isOutput=False)
    io["xv"] = nc.declare_dram_parameter("xv", [S, D], F32, isOutput=False)
    io["wq"] = nc.declare_dram_parameter("wq", [128, DC, EG], F16, isOutput=False)
    io["wk"] = nc.declare_dram_parameter("wk", [128, DC, EG], F16, isOutput=False)
    io["wv"] = nc.declare_dram_parameter("wv", [128, DC, EG], F16, isOutput=False)
    io["wo"] = nc.declare_dram_parameter("wo", [64, HG, D], F16, isOutput=False)
    io["bq"] = nc.declare_dram_parameter("bq", [128, 2], F32, isOutput=False)
    io["bk"] = nc.declare_dram_parameter("bk", [128, 2], F32, isOutput=False)
    io["out"] = nc.declare_dram_parameter("out", [L, D], F32, isOutput=True)
    with tile.TileContext(nc) as tc:
        with ExitStack() as ctx:
            _emit(ctx, tc, io)
    nc.compile()
    return nc


_NC = None


def _get_nc():
    global _NC
    if _NC is None:
        _NC = build_nc()
    return _NC


def _chunk_w(w):
    """[512, n] -> [128, 4, n] fp16:  [p, c, :] = w[128c+p, :]"""
    n = w.shape[1]
    return np.ascontiguousarray(
        w.reshape(DC, 128, n).transpose(1, 0, 2), dtype=np.float16)


def make_in_maps(queries, keys, values, tau, Wq, bq, Wk, bk, Wv, bv, Wo):
    in_maps = []
    for c in range(N_CORES):
        b, g = c // 2, c % 2
        e0 = g * EG
        f = np.float32(SCALE * tau[b])
        wq = _chunk_w(Wq[:, e0:e0 + EG] * f)
        wk = _chunk_w(Wk[:, e0:e0 + EG])
        wv = _chunk_w(Wv[:, e0:e0 + EG])
        wo = np.ascontiguousarray(
            Wo[e0:e0 + EG, :].reshape(HG, 64, D).transpose(1, 0, 2),
            dtype=np.float16)
        in_maps.append({
            "xq": np.ascontiguousarray(queries[b], dtype=np.float32),
            "xk": np.ascontiguousarray(keys[b], dtype=np.float32),
            "xv": np.ascontiguousarray(values[b], dtype=np.float32),
            "wq": wq, "wk": wk, "wv": wv, "wo": wo,
            "bq": np.ascontiguousarray(
                (bq[e0:e0 + EG] * f).reshape(2, 128).T, dtype=np.float32),
            "bk": np.ascontiguousarray(
                bk[e0:e0 + EG].reshape(2, 128).T, dtype=np.float32),
        })
    return in_maps


def kernel(queries, keys, values, tau, delta, Wq, bq, Wk, bk, Wv, bv, Wo, bo,
           **_unused):
    queries = np.asarray(queries, dtype=np.float32)
    keys = np.asarray(keys, dtype=np.float32)
    values = np.asarray(values, dtype=np.float32)
    tau = np.asarray(tau, dtype=np.float32)
    Wq, bq = np.asarray(Wq, np.float32), np.asarray(bq, np.float32)
    Wk, bk = np.asarray(Wk, np.float32), np.asarray(bk, np.float32)
    Wv, bv = np.asarray(Wv, np.float32), np.asarray(bv, np.float32)
    Wo, bo = np.asarray(Wo, np.float32), np.asarray(bo, np.float32)

    nc = _get_nc()
    in_maps = make_in_maps(queries, keys, values, tau, Wq, bq, Wk, bk, Wv, bv, Wo)
    res = run_bass_kernel_spmd(nc, in_maps, list(range(N_CORES)))
    # attn rows sum to 1 -> +bv flows through Wo as a constant row; + bo.
    const_row = (bv @ Wo + bo).astype(np.float32)  # [512]
    out = np.empty((B, L, D), dtype=np.float32)
    for b in range(B):
        out[b] = res.results[2 * b]["out"] + res.results[2 * b + 1]["out"] \
            + const_row
    return out


if __name__ == "__main__":
    nc = build_nc()
    print("built OK")
